# revision 21
# baseline (speedup 1.0000x reference)
"""BoxBlur 7x7 (normalized, reflect padding) on 8 Trainium2 NeuronCores.

Strategy (pure data parallel, 4 images x 3 channels = 12 image-planes per core):
  - Load-once geometry: the host prepends 3 zero rows to each core's
    [12288, 1024] row block, so the horizontal-pass tiles live on a grid
    shifted by -3 rows (h_tile t = x rows 128t-3 .. 128t+124).  Every input
    row is DMA'd exactly once, as uniform 1 MB batched loads (2 tiles per
    dma_start, contiguous in DRAM) on the sync HWDGE ring.
  - Horizontal 7-tap box sum per 128-row tile on VectorE: one rolling scan
    (tensor_tensor_scan, fp32 state) h[c] = h[c-1] + p[c+7] - p[c] over a
    reflect-padded row buffer, downcast to bf16 h.
  - Vertical 7-tap weighted sum on TensorE in bf16 (1 cycle/row): out_tile
    u [128, 1024] accumulates two banded matmuls per 512-column PSUM bank:
    W_a[128,128].T @ h_u + W_b[6,128].T @ h_{u+1}[0:6].  The bf16 band
    matrices carry the 1/49 normalization and the vertical reflect folding
    at each image-plane top/bottom (3 tile kinds: top/interior/bottom).
  - PSUM evacuated to an SBUF staging buffer on ScalarE (output stays
    fp32); stores go out as batched 1 MB SWDGE transfers on the gpsimd
    queue (>=4 KB SDMA descriptors, vs 528 B unbatched).
"""

import numpy as np

import concourse.bass as bass
import concourse.tile as tile
from concourse import bacc, mybir
from concourse.bass_utils import run_bass_kernel_spmd

H = W = 1024
KH = KW = 7
PAD = 3              # k // 2
SEG = 128            # h-tile / out-tile row count
N_CORES = 8
IMGS_PER_CORE = 4    # 32 / 8
CHANNELS = 3
IC_PER_CORE = IMGS_PER_CORE * CHANNELS       # 12 image-planes per core
ROWS = IC_PER_CORE * H                       # 12288
XROWS = ROWS + PAD                           # 12291 (3 zero rows on top)
N_TILES = ROWS // SEG                        # 96 h tiles (+1 runt) = out tiles
TPP = H // SEG                               # 8 tiles per plane

# padded row buffer: col 0 = 0.0, cols 1..3 = left reflect, 4..1027 = x,
# 1028..1030 = right reflect.
PBUF = W + KW        # 1031 valid columns
PBUF_ALLOC = 1032    # round to 8B

_F32 = mybir.dt.float32
_MM_DT = {
    "bf16": mybir.dt.bfloat16,
    "f32r": mybir.dt.float32r,
    "f32": mybir.dt.float32,
}

LOAD_ENGINES = ["sync"]            # HWDGE ring for input loads (keep ACT free)
STORE_ENGINES = ["gpsimd"]         # SWDGE spreads stores across 16 SDMA engines
LOAD_BATCH = 2                     # h tiles per load dma (1 MB)
STORE_BATCH = 2                    # out tiles per store dma (1 MB)
# bf16: 1 cycle/row + 1024-wide moving operand (one matmul per tile pair);
# h and the band weights are bf16, PSUM accumulation stays fp32.
MM_DTYPE = "bf16"                  # h / matmul moving-operand dtype
W_DTYPE = "bf16"                   # must match MM_DTYPE 32-bitness (walrus check)
SCAN_OFFLOAD = 0                   # every Nth h tile scans on gpsimd (0 = off)
HBUFS = 10
XBUFS = 6
OBUFS = 4
PSUM_BUFS = 4

_compiled = None  # cached compiled Bass program


def _dma_eng(nc, names, i):
    return getattr(nc, names[i % len(names)])


def _build_weights(kcol):
    """Band matrices for the vertical pass.

    h_tile t partition j corresponds to x row 128t + j - PAD (plane-local
    row (128t + j - PAD) % H).  Out tile u row m = x row 128u + m.  Returns
    wa_top/wa_int/wa_bot [128,128] and wb_int [6,128] / wb_bot [3,128]
    (wb applies to h_{u+1}; plane-bottom tiles only touch its first 3 rows).
    """
    def build(pos):
        Wa = np.zeros((SEG, SEG), np.float32)
        Wb = np.zeros((KW - 1, SEG), np.float32)
        for m in range(SEG):
            r_loc = pos * SEG + m            # plane-local out row
            for d in range(-PAD, PAD + 1):
                r = r_loc + d
                if r < 0:
                    r = -r
                if r > H - 1:
                    r = 2 * (H - 1) - r
                j = (r - pos * SEG) + PAD    # partition in h grid space
                if j < SEG:
                    Wa[j, m] += kcol[d + PAD]
                else:
                    Wb[j - SEG, m] += kcol[d + PAD]
        return Wa, Wb

    wa_top, wb_top = build(0)
    wa_int, wb_int = build(3)
    wa_bot, wb_bot = build(TPP - 1)
    assert np.array_equal(wb_top, wb_int)
    assert not wb_bot[PAD:].any()
    return wa_top, wa_int, wa_bot, wb_int, wb_bot[:PAD]


def _hscan(tc, nc, hpool, spool, xview, P, tag, eng=None):
    """Horizontal 7-tap sum of one padded row tile -> h tile [P, W]."""
    eng = eng or nc.vector
    mm_dt = _MM_DT[MM_DTYPE]
    h = hpool.tile([128, W], mm_dt, tag=tag)
    init = spool.tile([128, 8], _F32, tag="S")
    # free-dim reduce is DVE-only (gpsimd reduces along partitions instead)
    nc.vector.tensor_reduce(
        init[0:P, 0:1], xview[0:P, 0:KW],
        axis=mybir.AxisListType.X, op=mybir.AluOpType.add,
    )
    eng.tensor_tensor_scan(
        h[0:P, :],
        xview[0:P, KW:PBUF],
        xview[0:P, 0:W],
        init[0:P, 0:1],
        op0=mybir.AluOpType.add,
        op1=mybir.AluOpType.subtract,
    )
    return h


def _pad_tile(nc, xview, P):
    """Fill the reflect columns of one padded row tile.  These stay on
    VectorE: gpsimd copies measured 8x slower (Q7 software) and sit on the
    load -> pad -> scan critical chain.

    Col 0 gets x[4] instead of 0: the scan's init includes p[0] and
    h[0] = init + p[7] - p[0] subtracts it again, so any finite value
    cancels exactly -- one 4-wide copy replaces memset + 3-wide copy."""
    # cols 0..3 = x[4],x[3],x[2],x[1] = buf cols 8,7,6,5
    nc.vector.tensor_copy(xview[0:P, 0:4], xview[0:P, 8:4:-1])
    # right reflect: cols 1028..1030 = x[1022..1020] = buf 1026,1025,1024
    nc.vector.tensor_copy(xview[0:P, 1028:1031], xview[0:P, 1026:1023:-1])


def _body(tc, nc, x, ws, out):
    mm_dt = _MM_DT[MM_DTYPE]
    with (
        tc.tile_pool(name="wpool", bufs=1) as wpool,
        tc.tile_pool(name="xpad", bufs=XBUFS) as xpool,
        tc.tile_pool(name="scan", bufs=8) as spool,
        tc.tile_pool(name="hbuf", bufs=HBUFS) as hpool,
        tc.tile_pool(name="psum", bufs=PSUM_BUFS, space="PSUM") as ppool,
        tc.tile_pool(name="osb", bufs=OBUFS) as opool,
    ):
        # --- weights ---
        w_dt = _MM_DT[W_DTYPE]
        wa_t = []
        for i, name in enumerate(("wa_top", "wa_int", "wa_bot")):
            t = wpool.tile([128, SEG], w_dt, tag=name)
            nc.sync.dma_start(t[:, :], ws[name])
            wa_t.append(t)
        wb_int_t = wpool.tile([128, SEG], w_dt, tag="wb_int")
        nc.sync.dma_start(wb_int_t[0 : KW - 1, :], ws["wb_int"])
        wb_bot_t = wpool.tile([128, SEG], w_dt, tag="wb_bot")
        nc.sync.dma_start(wb_bot_t[0:PAD, :], ws["wb_bot"])

        # --- runt h tile (last 3 rows of the core block), computed once ---
        xr = xpool.tile([128, PBUF_ALLOC], _F32, tag="xr", bufs=1)
        nc.sync.dma_start(
            xr[0:PAD, KW - PAD : KW - PAD + W], x[N_TILES * SEG : XROWS, :]
        )
        _pad_tile(nc, xr, PAD)
        h_runt = _hscan(tc, nc, hpool, spool, xr, PAD, "hrunt")

        n_batches = N_TILES // LOAD_BATCH
        h_tiles = {N_TILES: h_runt}
        obuf = None

        def emit_out(u):
            nonlocal obuf
            pos = u % TPP
            kind = 0 if pos == 0 else (2 if pos == TPP - 1 else 1)
            wa = wa_t[kind]
            wb, k2 = (wb_bot_t, PAD) if kind == 2 else (wb_int_t, KW - 1)
            h_u = h_tiles[u]
            h_n = h_tiles[u + 1]
            ps = ppool.tile([128, W], _F32, tag="ps")
            # fp32 PSUM output maxes at 512 columns (one bank) per matmul.
            n_half = 2
            for half in range(n_half):
                sl = slice(half * (W // n_half), (half + 1) * (W // n_half))
                nc.tensor.matmul(
                    ps[:, sl], wa[:, :], h_u[:, sl], start=True, stop=False
                )
                nc.tensor.matmul(
                    ps[:, sl], wb[0:k2, :], h_n[0:k2, sl], start=False, stop=True
                )
            if u % STORE_BATCH == 0:
                obuf = opool.tile([128, STORE_BATCH * W], _F32, tag="ob")
            nc.scalar.copy(
                obuf[:, (u % STORE_BATCH) * W : (u % STORE_BATCH + 1) * W],
                ps[:, :],
            )
            if u % STORE_BATCH == STORE_BATCH - 1:
                sb = u // STORE_BATCH
                rows = STORE_BATCH * SEG
                dview = out[sb * rows : (sb + 1) * rows, :].rearrange(
                    "(s p) w -> p s w", s=STORE_BATCH
                )
                sview = obuf[:, :].rearrange("p (s w) -> p s w", s=STORE_BATCH)
                _dma_eng(nc, STORE_ENGINES, sb).dma_start(dview, sview)
            del h_tiles[u]

        for bt in range(n_batches):
            xb = xpool.tile([128, LOAD_BATCH * PBUF_ALLOC], _F32, tag="xb")
            rows = LOAD_BATCH * SEG
            dview = x[bt * rows : (bt + 1) * rows, :].rearrange(
                "(s p) w -> p s w", s=LOAD_BATCH
            )
            sview = xb[:, :].rearrange("p (s b) -> p s b", s=LOAD_BATCH)[
                :, :, KW - PAD : KW - PAD + W
            ]
            _dma_eng(nc, LOAD_ENGINES, bt).dma_start(sview, dview)
            for j in range(LOAD_BATCH):
                t = bt * LOAD_BATCH + j
                xv = xb[:, j * PBUF_ALLOC : (j + 1) * PBUF_ALLOC]
                _pad_tile(nc, xv, 128)
                eng = (
                    nc.gpsimd
                    if SCAN_OFFLOAD and t % SCAN_OFFLOAD == SCAN_OFFLOAD - 2
                    else nc.vector
                )
                h_tiles[t] = _hscan(tc, nc, hpool, spool, xv, 128, "h", eng=eng)
                if t >= 1:
                    emit_out(t - 1)
        emit_out(N_TILES - 1)


def _bass_program(num_devices=N_CORES):
    nc = bacc.Bacc(
        "TRN2",
        target_bir_lowering=False,
        debug=False,
        enable_asserts=False,
        num_devices=num_devices,
    )
    mm_dt = _MM_DT[MM_DTYPE]
    x_ap = nc.dram_tensor("x", [XROWS, W], _F32, kind="ExternalInput").ap()
    ws = {}
    for name, shape in (
        ("wa_top", [SEG, SEG]),
        ("wa_int", [SEG, SEG]),
        ("wa_bot", [SEG, SEG]),
        ("wb_int", [KW - 1, SEG]),
        ("wb_bot", [PAD, SEG]),
    ):
        ws[name] = nc.dram_tensor(
            name, shape, _MM_DT[W_DTYPE], kind="ExternalInput"
        ).ap()
    out_ap = nc.dram_tensor("out", [ROWS, W], _F32, kind="ExternalOutput").ap()
    with tile.TileContext(nc) as tc:
        _body(tc, nc, x_ap, ws, out_ap)
    nc.compile()
    return nc


def _get_program():
    global _compiled
    if _compiled is None:
        _compiled = _bass_program()
    return _compiled


def _make_in_maps(x, kernel):
    x = np.ascontiguousarray(np.asarray(x, dtype=np.float32))
    assert x.shape == (N_CORES * IMGS_PER_CORE, CHANNELS, H, W), x.shape
    k2 = np.asarray(kernel, dtype=np.float64)
    k2 = k2 / k2.sum()
    # horizontal pass is an unweighted 7-tap sum => all columns of the
    # normalized kernel must be identical (true for the box kernel).
    assert np.allclose(k2, k2[:, :1]), "kernel must have uniform rows"
    kcol = k2[:, 0].astype(np.float32)
    wa_top, wa_int, wa_bot, wb_int, wb_bot = _build_weights(kcol)
    if W_DTYPE == "bf16":
        import ml_dtypes

        wa_top, wa_int, wa_bot, wb_int, wb_bot = (
            w.astype(ml_dtypes.bfloat16)
            for w in (wa_top, wa_int, wa_bot, wb_int, wb_bot)
        )
    xr = x.reshape(N_CORES, ROWS, W)
    xp = np.zeros((N_CORES, XROWS, W), np.float32)
    xp[:, PAD:, :] = xr
    return [
        {
            "x": xp[c],
            "wa_top": wa_top,
            "wa_int": wa_int,
            "wa_bot": wa_bot,
            "wb_int": wb_int,
            "wb_bot": wb_bot,
        }
        for c in range(N_CORES)
    ]


def run_shards(in_maps, **kwargs):
    """Compile (cached) + run on cores 0..7; returns BassKernelResults."""
    nc = _get_program()
    return run_bass_kernel_spmd(nc, in_maps, core_ids=list(range(N_CORES)), **kwargs)


def kernel(x, kernel):
    in_maps = _make_in_maps(x, kernel)
    try:
        res = run_shards(in_maps)
    except Exception:
        # one retry: transient NRT device errors have been observed under
        # the PJRT/axon path; the device recovers on a fresh dispatch.
        import time as _time

        _time.sleep(30)
        res = run_shards(in_maps)
    outs = [
        res.results[c]["out"].reshape(IMGS_PER_CORE, CHANNELS, H, W)
        for c in range(N_CORES)
    ]
    return np.concatenate(outs, axis=0)


# revision 23
# speedup vs baseline: 1.0066x; 1.0066x over previous
"""BoxBlur 7x7 (normalized, reflect padding) on 8 Trainium2 NeuronCores.

Strategy (pure data parallel, 4 images x 3 channels = 12 image-planes per core):
  - Load-once geometry: the host prepends 3 zero rows to each core's
    [12288, 1024] row block, so the horizontal-pass tiles live on a grid
    shifted by -3 rows (h_tile t = x rows 128t-3 .. 128t+124).  Every input
    row is DMA'd exactly once, as uniform 1 MB batched loads (2 tiles per
    dma_start, contiguous in DRAM) on the sync HWDGE ring.
  - Horizontal 7-tap box sum per 128-row tile on VectorE: one rolling scan
    (tensor_tensor_scan, fp32 state) h[c] = h[c-1] + p[c+7] - p[c] over a
    reflect-padded row buffer, downcast to bf16 h.
  - Vertical 7-tap weighted sum on TensorE in bf16 (1 cycle/row): out_tile
    u [128, 1024] accumulates two banded matmuls per 512-column PSUM bank:
    W_a[128,128].T @ h_u + W_b[6,128].T @ h_{u+1}[0:6].  The bf16 band
    matrices carry the 1/49 normalization and the vertical reflect folding
    at each image-plane top/bottom (3 tile kinds: top/interior/bottom).
  - PSUM evacuated to an SBUF staging buffer on ScalarE (output stays
    fp32); stores go out as batched 1 MB SWDGE transfers on the gpsimd
    queue (>=4 KB SDMA descriptors, vs 528 B unbatched).
"""

import numpy as np

import concourse.bass as bass
import concourse.tile as tile
from concourse import bacc, mybir
from concourse.bass_utils import run_bass_kernel_spmd

H = W = 1024
KH = KW = 7
PAD = 3              # k // 2
SEG = 128            # h-tile / out-tile row count
N_CORES = 8
IMGS_PER_CORE = 4    # 32 / 8
CHANNELS = 3
IC_PER_CORE = IMGS_PER_CORE * CHANNELS       # 12 image-planes per core
ROWS = IC_PER_CORE * H                       # 12288
XROWS = ROWS + PAD                           # 12291 (3 zero rows on top)
N_TILES = ROWS // SEG                        # 96 h tiles (+1 runt) = out tiles
TPP = H // SEG                               # 8 tiles per plane

# padded row buffer: col 0 = 0.0, cols 1..3 = left reflect, 4..1027 = x,
# 1028..1030 = right reflect.
PBUF = W + KW        # 1031 valid columns
PBUF_ALLOC = 1032    # round to 8B

_F32 = mybir.dt.float32
_MM_DT = {
    "bf16": mybir.dt.bfloat16,
    "f32r": mybir.dt.float32r,
    "f32": mybir.dt.float32,
}

LOAD_ENGINES = ["sync"]            # HWDGE ring for input loads (keep ACT free)
STORE_ENGINES = ["gpsimd"]         # SWDGE spreads stores across 16 SDMA engines
LOAD_BATCH = 2                     # h tiles per load dma (1 MB)
STORE_BATCH = 2                    # out tiles per store dma (1 MB)
# bf16: 1 cycle/row + 1024-wide moving operand (one matmul per tile pair);
# h and the band weights are bf16, PSUM accumulation stays fp32.
MM_DTYPE = "bf16"                  # h / matmul moving-operand dtype
W_DTYPE = "bf16"                   # must match MM_DTYPE 32-bitness (walrus check)
SCAN_OFFLOAD = 0                   # every Nth h tile scans on gpsimd (0 = off)
HBUFS = 10
XBUFS = 6
OBUFS = 4
PSUM_BUFS = 4

_compiled = None  # cached compiled Bass program


def _dma_eng(nc, names, i):
    return getattr(nc, names[i % len(names)])


def _build_weights(kcol):
    """Band matrices for the vertical pass.

    h_tile t partition j corresponds to x row 128t + j - PAD (plane-local
    row (128t + j - PAD) % H).  Out tile u row m = x row 128u + m.  Returns
    wa_top/wa_int/wa_bot [128,128] and wb_int [6,128] / wb_bot [3,128]
    (wb applies to h_{u+1}; plane-bottom tiles only touch its first 3 rows).
    """
    def build(pos):
        Wa = np.zeros((SEG, SEG), np.float32)
        Wb = np.zeros((KW - 1, SEG), np.float32)
        for m in range(SEG):
            r_loc = pos * SEG + m            # plane-local out row
            for d in range(-PAD, PAD + 1):
                r = r_loc + d
                if r < 0:
                    r = -r
                if r > H - 1:
                    r = 2 * (H - 1) - r
                j = (r - pos * SEG) + PAD    # partition in h grid space
                if j < SEG:
                    Wa[j, m] += kcol[d + PAD]
                else:
                    Wb[j - SEG, m] += kcol[d + PAD]
        return Wa, Wb

    wa_top, wb_top = build(0)
    wa_int, wb_int = build(3)
    wa_bot, wb_bot = build(TPP - 1)
    assert np.array_equal(wb_top, wb_int)
    assert not wb_bot[PAD:].any()
    return wa_top, wa_int, wa_bot, wb_int, wb_bot[:PAD]


def _hscan(tc, nc, hpool, spool, xview, P, tag, eng=None):
    """Horizontal 7-tap sum of one padded row tile -> h tile [P, W]."""
    eng = eng or nc.vector
    mm_dt = _MM_DT[MM_DTYPE]
    h = hpool.tile([128, W], mm_dt, tag=tag)
    init = spool.tile([128, 8], _F32, tag="S")
    # free-dim reduce is DVE-only (gpsimd reduces along partitions instead)
    nc.vector.tensor_reduce(
        init[0:P, 0:1], xview[0:P, 0:KW],
        axis=mybir.AxisListType.X, op=mybir.AluOpType.add,
    )
    eng.tensor_tensor_scan(
        h[0:P, :],
        xview[0:P, KW:PBUF],
        xview[0:P, 0:W],
        init[0:P, 0:1],
        op0=mybir.AluOpType.add,
        op1=mybir.AluOpType.subtract,
    )
    return h


def _pad_tile(nc, xview, P):
    """Fill memset + reflect columns of one padded row tile.  These stay on
    VectorE: gpsimd copies measured 8x slower (Q7 software) and sit on the
    load -> pad -> scan critical chain.  (Dropping the memset by widening
    the left copy is numerically valid -- p[0] cancels in the scan -- but
    measured 327us vs 271us: the reshuffled static schedule lost more to
    Vector idle gaps than the 97 removed instructions saved.)"""
    nc.vector.memset(xview[0:P, 0:1], 0.0)
    # left reflect: cols 1..3 = x[3],x[2],x[1] = buf cols 7,6,5
    nc.vector.tensor_copy(xview[0:P, 1:4], xview[0:P, 7:4:-1])
    # right reflect: cols 1028..1030 = x[1022..1020] = buf 1026,1025,1024
    nc.vector.tensor_copy(xview[0:P, 1028:1031], xview[0:P, 1026:1023:-1])


def _body(tc, nc, x, ws, out):
    mm_dt = _MM_DT[MM_DTYPE]
    with (
        tc.tile_pool(name="wpool", bufs=1) as wpool,
        tc.tile_pool(name="xpad", bufs=XBUFS) as xpool,
        tc.tile_pool(name="scan", bufs=8) as spool,
        tc.tile_pool(name="hbuf", bufs=HBUFS) as hpool,
        tc.tile_pool(name="psum", bufs=PSUM_BUFS, space="PSUM") as ppool,
        tc.tile_pool(name="osb", bufs=OBUFS) as opool,
    ):
        # --- weights ---
        w_dt = _MM_DT[W_DTYPE]
        wa_t = []
        for i, name in enumerate(("wa_top", "wa_int", "wa_bot")):
            t = wpool.tile([128, SEG], w_dt, tag=name)
            nc.sync.dma_start(t[:, :], ws[name])
            wa_t.append(t)
        wb_int_t = wpool.tile([128, SEG], w_dt, tag="wb_int")
        nc.sync.dma_start(wb_int_t[0 : KW - 1, :], ws["wb_int"])
        wb_bot_t = wpool.tile([128, SEG], w_dt, tag="wb_bot")
        nc.sync.dma_start(wb_bot_t[0:PAD, :], ws["wb_bot"])

        n_batches = N_TILES // LOAD_BATCH
        h_tiles = {}
        obuf = None

        def emit_out(u):
            nonlocal obuf
            pos = u % TPP
            kind = 0 if pos == 0 else (2 if pos == TPP - 1 else 1)
            wa = wa_t[kind]
            wb, k2 = (wb_bot_t, PAD) if kind == 2 else (wb_int_t, KW - 1)
            h_u = h_tiles[u]
            h_n = h_tiles[u + 1]
            ps = ppool.tile([128, W], _F32, tag="ps")
            # fp32 PSUM output maxes at 512 columns (one bank) per matmul.
            n_half = 2
            for half in range(n_half):
                sl = slice(half * (W // n_half), (half + 1) * (W // n_half))
                nc.tensor.matmul(
                    ps[:, sl], wa[:, :], h_u[:, sl], start=True, stop=False
                )
                nc.tensor.matmul(
                    ps[:, sl], wb[0:k2, :], h_n[0:k2, sl], start=False, stop=True
                )
            if u % STORE_BATCH == 0:
                obuf = opool.tile([128, STORE_BATCH * W], _F32, tag="ob")
            nc.scalar.copy(
                obuf[:, (u % STORE_BATCH) * W : (u % STORE_BATCH + 1) * W],
                ps[:, :],
            )
            if u % STORE_BATCH == STORE_BATCH - 1:
                sb = u // STORE_BATCH
                rows = STORE_BATCH * SEG
                dview = out[sb * rows : (sb + 1) * rows, :].rearrange(
                    "(s p) w -> p s w", s=STORE_BATCH
                )
                sview = obuf[:, :].rearrange("p (s w) -> p s w", s=STORE_BATCH)
                _dma_eng(nc, STORE_ENGINES, sb).dma_start(dview, sview)
            del h_tiles[u]

        for bt in range(n_batches):
            xb = xpool.tile([128, LOAD_BATCH * PBUF_ALLOC], _F32, tag="xb")
            rows = LOAD_BATCH * SEG
            dview = x[bt * rows : (bt + 1) * rows, :].rearrange(
                "(s p) w -> p s w", s=LOAD_BATCH
            )
            sview = xb[:, :].rearrange("p (s b) -> p s b", s=LOAD_BATCH)[
                :, :, KW - PAD : KW - PAD + W
            ]
            _dma_eng(nc, LOAD_ENGINES, bt).dma_start(sview, dview)
            for j in range(LOAD_BATCH):
                t = bt * LOAD_BATCH + j
                xv = xb[:, j * PBUF_ALLOC : (j + 1) * PBUF_ALLOC]
                _pad_tile(nc, xv, 128)
                eng = (
                    nc.gpsimd
                    if SCAN_OFFLOAD and t % SCAN_OFFLOAD == SCAN_OFFLOAD - 2
                    else nc.vector
                )
                h_tiles[t] = _hscan(tc, nc, hpool, spool, xv, 128, "h", eng=eng)
                if t >= 1:
                    emit_out(t - 1)
            if bt == 1:
                # runt h tile (last 3 rows of the core block): only needed by
                # out tile 95, so keep it off the warm-up critical path
                xr = xpool.tile([128, PBUF_ALLOC], _F32, tag="xr", bufs=1)
                nc.sync.dma_start(
                    xr[0:PAD, KW - PAD : KW - PAD + W],
                    x[N_TILES * SEG : XROWS, :],
                )
                _pad_tile(nc, xr, PAD)
                h_tiles[N_TILES] = _hscan(tc, nc, hpool, spool, xr, PAD, "hrunt")
        emit_out(N_TILES - 1)


def _bass_program(num_devices=N_CORES):
    nc = bacc.Bacc(
        "TRN2",
        target_bir_lowering=False,
        debug=False,
        enable_asserts=False,
        num_devices=num_devices,
    )
    mm_dt = _MM_DT[MM_DTYPE]
    x_ap = nc.dram_tensor("x", [XROWS, W], _F32, kind="ExternalInput").ap()
    ws = {}
    for name, shape in (
        ("wa_top", [SEG, SEG]),
        ("wa_int", [SEG, SEG]),
        ("wa_bot", [SEG, SEG]),
        ("wb_int", [KW - 1, SEG]),
        ("wb_bot", [PAD, SEG]),
    ):
        ws[name] = nc.dram_tensor(
            name, shape, _MM_DT[W_DTYPE], kind="ExternalInput"
        ).ap()
    out_ap = nc.dram_tensor("out", [ROWS, W], _F32, kind="ExternalOutput").ap()
    with tile.TileContext(nc) as tc:
        _body(tc, nc, x_ap, ws, out_ap)
    nc.compile()
    return nc


def _get_program():
    global _compiled
    if _compiled is None:
        _compiled = _bass_program()
    return _compiled


def _make_in_maps(x, kernel):
    x = np.ascontiguousarray(np.asarray(x, dtype=np.float32))
    assert x.shape == (N_CORES * IMGS_PER_CORE, CHANNELS, H, W), x.shape
    k2 = np.asarray(kernel, dtype=np.float64)
    k2 = k2 / k2.sum()
    # horizontal pass is an unweighted 7-tap sum => all columns of the
    # normalized kernel must be identical (true for the box kernel).
    assert np.allclose(k2, k2[:, :1]), "kernel must have uniform rows"
    kcol = k2[:, 0].astype(np.float32)
    wa_top, wa_int, wa_bot, wb_int, wb_bot = _build_weights(kcol)
    if W_DTYPE == "bf16":
        import ml_dtypes

        wa_top, wa_int, wa_bot, wb_int, wb_bot = (
            w.astype(ml_dtypes.bfloat16)
            for w in (wa_top, wa_int, wa_bot, wb_int, wb_bot)
        )
    xr = x.reshape(N_CORES, ROWS, W)
    xp = np.zeros((N_CORES, XROWS, W), np.float32)
    xp[:, PAD:, :] = xr
    return [
        {
            "x": xp[c],
            "wa_top": wa_top,
            "wa_int": wa_int,
            "wa_bot": wa_bot,
            "wb_int": wb_int,
            "wb_bot": wb_bot,
        }
        for c in range(N_CORES)
    ]


def run_shards(in_maps, **kwargs):
    """Compile (cached) + run on cores 0..7; returns BassKernelResults."""
    nc = _get_program()
    return run_bass_kernel_spmd(nc, in_maps, core_ids=list(range(N_CORES)), **kwargs)


def kernel(x, kernel):
    in_maps = _make_in_maps(x, kernel)
    try:
        res = run_shards(in_maps)
    except Exception:
        # one retry: transient NRT device errors have been observed under
        # the PJRT/axon path; the device recovers on a fresh dispatch.
        import time as _time

        _time.sleep(30)
        res = run_shards(in_maps)
    outs = [
        res.results[c]["out"].reshape(IMGS_PER_CORE, CHANNELS, H, W)
        for c in range(N_CORES)
    ]
    return np.concatenate(outs, axis=0)


# revision 25
# speedup vs baseline: 1.0992x; 1.0920x over previous
"""BoxBlur 7x7 (normalized, reflect padding) on 8 Trainium2 NeuronCores.

Strategy (pure data parallel, 4 images x 3 channels = 12 image-planes per core):
  - Load-once geometry: the host prepends 3 zero rows to each core's
    [12288, 1024] row block, so the horizontal-pass tiles live on a grid
    shifted by -3 rows (h_tile t = x rows 128t-3 .. 128t+124).  Every input
    row is DMA'd exactly once, as uniform 1 MB batched loads (2 tiles per
    dma_start, contiguous in DRAM) on the sync HWDGE ring.
  - Horizontal 7-tap box sum per 128-row tile on VectorE: one rolling scan
    (tensor_tensor_scan, fp32 state) h[c] = h[c-1] + p[c+7] - p[c] over a
    reflect-padded row buffer, downcast to bf16 h.
  - Vertical 7-tap weighted sum on TensorE in bf16 (1 cycle/row): out_tile
    u [128, 1024] accumulates two banded matmuls per 512-column PSUM bank:
    W_a[128,128].T @ h_u + W_b[6,128].T @ h_{u+1}[0:6].  The bf16 band
    matrices carry the 1/49 normalization and the vertical reflect folding
    at each image-plane top/bottom (3 tile kinds: top/interior/bottom).
  - PSUM evacuated to an SBUF staging buffer on ScalarE (output stays
    fp32); stores go out as batched 1 MB SWDGE transfers on the gpsimd
    queue (>=4 KB SDMA descriptors, vs 528 B unbatched).
"""

import numpy as np

import concourse.bass as bass
import concourse.tile as tile
from concourse import bacc, mybir
from concourse.bass_utils import run_bass_kernel_spmd

H = W = 1024
KH = KW = 7
PAD = 3              # k // 2
SEG = 128            # h-tile / out-tile row count
N_CORES = 8
IMGS_PER_CORE = 4    # 32 / 8
CHANNELS = 3
IC_PER_CORE = IMGS_PER_CORE * CHANNELS       # 12 image-planes per core
ROWS = IC_PER_CORE * H                       # 12288
XROWS = ROWS + PAD                           # 12291 (3 zero rows on top)
N_TILES = ROWS // SEG                        # 96 h tiles (+1 runt) = out tiles
TPP = H // SEG                               # 8 tiles per plane

# padded row buffer: col 0 = 0.0, cols 1..3 = left reflect, 4..1027 = x,
# 1028..1030 = right reflect.
PBUF = W + KW        # 1031 valid columns
PBUF_ALLOC = 1032    # round to 8B

_F32 = mybir.dt.float32
_MM_DT = {
    "bf16": mybir.dt.bfloat16,
    "f32r": mybir.dt.float32r,
    "f32": mybir.dt.float32,
}

LOAD_ENGINES = ["sync"]            # HWDGE ring for input loads (keep ACT free)
STORE_ENGINES = ["gpsimd"]         # SWDGE spreads stores across 16 SDMA engines
LOAD_BATCH = 2                     # h tiles per load dma (1 MB)
STORE_BATCH = 2                    # out tiles per store dma (1 MB)
# bf16: 1 cycle/row + 1024-wide moving operand (one matmul per tile pair);
# h and the band weights are bf16, PSUM accumulation stays fp32.
MM_DTYPE = "bf16"                  # h / matmul moving-operand dtype
W_DTYPE = "bf16"                   # must match MM_DTYPE 32-bitness (walrus check)
SCAN_OFFLOAD = 0                   # every Nth h tile scans on gpsimd (0 = off)
HBUFS = 10
XBUFS = 6
OBUFS = 4
PSUM_BUFS = 4

_compiled = None  # cached compiled Bass program


def _dma_eng(nc, names, i):
    return getattr(nc, names[i % len(names)])


def _build_weights(kcol):
    """Band matrices for the vertical pass.

    h_tile t partition j corresponds to x row 128t + j - PAD (plane-local
    row (128t + j - PAD) % H).  Out tile u row m = x row 128u + m.  Returns
    wa_top/wa_int/wa_bot [128,128] and wb_int [6,128] / wb_bot [3,128]
    (wb applies to h_{u+1}; plane-bottom tiles only touch its first 3 rows).
    """
    def build(pos):
        Wa = np.zeros((SEG, SEG), np.float32)
        Wb = np.zeros((KW - 1, SEG), np.float32)
        for m in range(SEG):
            r_loc = pos * SEG + m            # plane-local out row
            for d in range(-PAD, PAD + 1):
                r = r_loc + d
                if r < 0:
                    r = -r
                if r > H - 1:
                    r = 2 * (H - 1) - r
                j = (r - pos * SEG) + PAD    # partition in h grid space
                if j < SEG:
                    Wa[j, m] += kcol[d + PAD]
                else:
                    Wb[j - SEG, m] += kcol[d + PAD]
        return Wa, Wb

    wa_top, wb_top = build(0)
    wa_int, wb_int = build(3)
    wa_bot, wb_bot = build(TPP - 1)
    assert np.array_equal(wb_top, wb_int)
    assert not wb_bot[PAD:].any()
    return wa_top, wa_int, wa_bot, wb_int, wb_bot[:PAD]


def _hscan(tc, nc, hpool, spool, xview, P, tag, eng=None):
    """Horizontal 7-tap sum of one padded row tile -> h tile [P, W]."""
    eng = eng or nc.vector
    mm_dt = _MM_DT[MM_DTYPE]
    h = hpool.tile([128, W], mm_dt, tag=tag)
    init = spool.tile([128, 8], _F32, tag="S")
    # free-dim reduce is DVE-only (gpsimd reduces along partitions instead)
    nc.vector.tensor_reduce(
        init[0:P, 0:1], xview[0:P, 0:KW],
        axis=mybir.AxisListType.X, op=mybir.AluOpType.add,
    )
    eng.tensor_tensor_scan(
        h[0:P, :],
        xview[0:P, KW:PBUF],
        xview[0:P, 0:W],
        init[0:P, 0:1],
        op0=mybir.AluOpType.add,
        op1=mybir.AluOpType.subtract,
    )
    return h


def _pad_tile(nc, xview, P):
    """Fill memset + reflect columns of one padded row tile.  These stay on
    VectorE: gpsimd copies measured 8x slower (Q7 software) and sit on the
    load -> pad -> scan critical chain.  (Dropping the memset by widening
    the left copy is numerically valid -- p[0] cancels in the scan -- but
    measured 327us vs 271us: the reshuffled static schedule lost more to
    Vector idle gaps than the 97 removed instructions saved.)"""
    nc.vector.memset(xview[0:P, 0:1], 0.0)
    # left reflect: cols 1..3 = x[3],x[2],x[1] = buf cols 7,6,5
    nc.vector.tensor_copy(xview[0:P, 1:4], xview[0:P, 7:4:-1])
    # right reflect: cols 1028..1030 = x[1022..1020] = buf 1026,1025,1024
    nc.vector.tensor_copy(xview[0:P, 1028:1031], xview[0:P, 1026:1023:-1])


def _body(tc, nc, x, ws, out):
    mm_dt = _MM_DT[MM_DTYPE]
    with (
        tc.tile_pool(name="wpool", bufs=1) as wpool,
        tc.tile_pool(name="xpad", bufs=XBUFS) as xpool,
        tc.tile_pool(name="scan", bufs=8) as spool,
        tc.tile_pool(name="hbuf", bufs=HBUFS) as hpool,
        tc.tile_pool(name="psum", bufs=PSUM_BUFS, space="PSUM") as ppool,
        tc.tile_pool(name="osb", bufs=OBUFS) as opool,
    ):
        # --- weights ---
        w_dt = _MM_DT[W_DTYPE]
        wa_t = []
        for i, name in enumerate(("wa_top", "wa_int", "wa_bot")):
            t = wpool.tile([128, SEG], w_dt, tag=name)
            nc.sync.dma_start(t[:, :], ws[name])
            wa_t.append(t)
        wb_int_t = wpool.tile([128, SEG], w_dt, tag="wb_int")
        nc.sync.dma_start(wb_int_t[0 : KW - 1, :], ws["wb_int"])
        wb_bot_t = wpool.tile([128, SEG], w_dt, tag="wb_bot")
        nc.sync.dma_start(wb_bot_t[0:PAD, :], ws["wb_bot"])

        n_batches = N_TILES // LOAD_BATCH
        h_tiles = {}
        obuf = None

        def emit_out(u):
            nonlocal obuf
            pos = u % TPP
            kind = 0 if pos == 0 else (2 if pos == TPP - 1 else 1)
            wa = wa_t[kind]
            wb, k2 = (wb_bot_t, PAD) if kind == 2 else (wb_int_t, KW - 1)
            h_u = h_tiles[u]
            h_n = h_tiles[u + 1]
            ps = ppool.tile([128, W], _F32, tag="ps")
            # fp32 PSUM output maxes at 512 columns (one bank) per matmul.
            n_half = 2
            for half in range(n_half):
                sl = slice(half * (W // n_half), (half + 1) * (W // n_half))
                nc.tensor.matmul(
                    ps[:, sl], wa[:, :], h_u[:, sl], start=True, stop=False
                )
                nc.tensor.matmul(
                    ps[:, sl], wb[0:k2, :], h_n[0:k2, sl], start=False, stop=True
                )
            if u % STORE_BATCH == 0:
                obuf = opool.tile([128, STORE_BATCH * W], _F32, tag="ob")
            nc.scalar.copy(
                obuf[:, (u % STORE_BATCH) * W : (u % STORE_BATCH + 1) * W],
                ps[:, :],
            )
            if u % STORE_BATCH == STORE_BATCH - 1:
                sb = u // STORE_BATCH
                rows = STORE_BATCH * SEG
                dview = out[sb * rows : (sb + 1) * rows, :].rearrange(
                    "(s p) w -> p s w", s=STORE_BATCH
                )
                sview = obuf[:, :].rearrange("p (s w) -> p s w", s=STORE_BATCH)
                _dma_eng(nc, STORE_ENGINES, sb).dma_start(dview, sview)
            del h_tiles[u]

        for bt in range(n_batches):
            xb = xpool.tile([128, LOAD_BATCH * PBUF_ALLOC], _F32, tag="xb")
            rows = LOAD_BATCH * SEG
            dview = x[bt * rows : (bt + 1) * rows, :].rearrange(
                "(s p) w -> p s w", s=LOAD_BATCH
            )
            sview = xb[:, :].rearrange("p (s b) -> p s b", s=LOAD_BATCH)[
                :, :, KW - PAD : KW - PAD + W
            ]
            _dma_eng(nc, LOAD_ENGINES, bt).dma_start(sview, dview)
            for j in range(LOAD_BATCH):
                t = bt * LOAD_BATCH + j
                xv = xb[:, j * PBUF_ALLOC : (j + 1) * PBUF_ALLOC]
                _pad_tile(nc, xv, 128)
                eng = (
                    nc.gpsimd
                    if SCAN_OFFLOAD and t % SCAN_OFFLOAD == SCAN_OFFLOAD - 2
                    else nc.vector
                )
                h_tiles[t] = _hscan(tc, nc, hpool, spool, xv, 128, "h", eng=eng)
                if t >= 1:
                    emit_out(t - 1)
            if bt == 1:
                # runt h tile (last 3 rows of the core block): only needed by
                # out tile 95, so keep it off the warm-up critical path
                xr = xpool.tile([128, PBUF_ALLOC], _F32, tag="xr", bufs=1)
                nc.sync.dma_start(
                    xr[0:PAD, KW - PAD : KW - PAD + W],
                    x[N_TILES * SEG : XROWS, :],
                )
                _pad_tile(nc, xr, PAD)
                h_tiles[N_TILES] = _hscan(tc, nc, hpool, spool, xr, PAD, "hrunt")
        emit_out(N_TILES - 1)


def _bass_program(num_devices=N_CORES):
    nc = bacc.Bacc(
        "TRN2",
        target_bir_lowering=False,
        debug=False,
        enable_asserts=False,
        num_devices=num_devices,
    )
    mm_dt = _MM_DT[MM_DTYPE]
    x_ap = nc.dram_tensor("x", [XROWS, W], _F32, kind="ExternalInput").ap()
    ws = {}
    for name, shape in (
        ("wa_top", [SEG, SEG]),
        ("wa_int", [SEG, SEG]),
        ("wa_bot", [SEG, SEG]),
        ("wb_int", [KW - 1, SEG]),
        ("wb_bot", [PAD, SEG]),
    ):
        ws[name] = nc.dram_tensor(
            name, shape, _MM_DT[W_DTYPE], kind="ExternalInput"
        ).ap()
    out_ap = nc.dram_tensor("out", [ROWS, W], _F32, kind="ExternalOutput").ap()
    with tile.TileContext(nc) as tc:
        _body(tc, nc, x_ap, ws, out_ap)
    nc.compile()
    return nc


def _get_program():
    global _compiled
    if _compiled is None:
        _compiled = _bass_program()
    return _compiled


def _make_in_maps(x, kernel):
    x = np.ascontiguousarray(np.asarray(x, dtype=np.float32))
    assert x.shape == (N_CORES * IMGS_PER_CORE, CHANNELS, H, W), x.shape
    k2 = np.asarray(kernel, dtype=np.float64)
    k2 = k2 / k2.sum()
    # horizontal pass is an unweighted 7-tap sum => all columns of the
    # normalized kernel must be identical (true for the box kernel).
    assert np.allclose(k2, k2[:, :1]), "kernel must have uniform rows"
    kcol = k2[:, 0].astype(np.float32)
    wa_top, wa_int, wa_bot, wb_int, wb_bot = _build_weights(kcol)
    if W_DTYPE == "bf16":
        import ml_dtypes

        wa_top, wa_int, wa_bot, wb_int, wb_bot = (
            w.astype(ml_dtypes.bfloat16)
            for w in (wa_top, wa_int, wa_bot, wb_int, wb_bot)
        )
    xr = x.reshape(N_CORES, ROWS, W)
    xp = np.zeros((N_CORES, XROWS, W), np.float32)
    xp[:, PAD:, :] = xr
    return [
        {
            "x": xp[c],
            "wa_top": wa_top,
            "wa_int": wa_int,
            "wa_bot": wa_bot,
            "wb_int": wb_int,
            "wb_bot": wb_bot,
        }
        for c in range(N_CORES)
    ]


def run_shards(in_maps, **kwargs):
    """Compile (cached) + run on cores 0..7; returns BassKernelResults."""
    nc = _get_program()
    return run_bass_kernel_spmd(nc, in_maps, core_ids=list(range(N_CORES)), **kwargs)


def kernel(x, kernel):
    in_maps = _make_in_maps(x, kernel)
    try:
        res = run_shards(in_maps)
    except Exception:
        # one retry: transient NRT device errors have been observed under
        # the PJRT/axon path; the device recovers on a fresh dispatch.
        import time as _time

        _time.sleep(30)
        res = run_shards(in_maps)
    outs = [
        res.results[c]["out"].reshape(IMGS_PER_CORE, CHANNELS, H, W)
        for c in range(N_CORES)
    ]
    return np.concatenate(outs, axis=0)
